# revision 59
# baseline (speedup 1.0000x reference)
"""Trainium2 Bass kernel: MHSA with multi-head relative position embedding.

Sharding: data-parallel over batch — 16 batches / 8 cores = 2 batches per core,
each core computes all 8 heads for its 2 batches. No collectives needed.

Math per batch (N=784 tokens, C=512, 8 heads x 64 dim):
  qkv = x @ w_qkv                  (q-columns pre-scaled by 1/8 on host)
  scores_T[k,q] = k_h^T q_h        (head pair packed in one 4-bank PSUM tile;
                                    even/odd head matmuls issued adjacently so
                                    they pack into disjoint PE row groups)
  E = exp(scores_T) * expbias_T    (ONE fused exp on ACT covering both heads'
                                    784-wide rows via a strided AP; bias
                                    multiply split DVE/GPSIMD, bias exp'd host)
  O_T[d,q] = sum_k v_aug[k,d] E[k,q]  with v_aug = [v | 1] -> row 64 = sumexp
  attnT = O_T[0:64] / O_T[64]      (denominator rows DMA-gathered, recip'd,
                                    replicated to 128 partitions with a
                                    stride-0 DMA, DVE mult)
  out = attnT^T stacked over heads @ w_out   (written bf16, host casts f32)

Perf structure vs v1: bias tables loaded ONCE (8 fat DMAs, resident in SBUF
for both batches) instead of per-batch per-tile; PE warmed with dummy matmuls
during the initial DMA wait so qkv runs at full clock (HAM 8/8); exp fused to
one ACTIVATE per (pair, kt) halving ACT instruction overhead; bias multiplies
split ~50/50 between DVE and GPSIMD; projection fillers redistributed so the
PE never idles >2us (keeps the HAM clock-gate warm).
"""

import numpy as np
import ml_dtypes

B, HH, WW, C = 16, 28, 28, 512
N = HH * WW            # 784 tokens
HEADS, KD = 8, 64
NCORES, BPC = 8, 2     # 8 cores, 2 batches per core
NT, TP = 7, 112        # 784 = 7 tiles of 112 (k / token tiling)
CHUNKS = [(0, 512), (512, 272)]   # q-chunks (PSUM bank = 512 fp32)
CT = 4                 # contraction tiles of 128 over C=512

_CACHE = {}


def _rel_index():
    # Faithful to reference._relative_position_index: token r -> (r%28, r//28)
    t = np.arange(N)
    c0, c1 = t % HH, t // HH
    return ((c0[:, None] - c0[None, :] + HH - 1)
            + (c1[:, None] - c1[None, :] + WW - 1) * (2 * HH - 1))  # [q, k]


def build_nc():
    if 'nc' in _CACHE:
        return _CACHE['nc']
    from contextlib import ExitStack
    import concourse.bacc as bacc
    import concourse.mybir as mybir
    import concourse.tile as tile
    from concourse.alu_op_type import AluOpType

    f32 = mybir.dt.float32
    bf16 = mybir.dt.bfloat16
    EXP = mybir.ActivationFunctionType.Exp

    nc = bacc.Bacc("TRN2", debug=False, enable_asserts=False)
    xT_d = nc.dram_tensor("xT", [BPC, C, N], bf16, kind="ExternalInput").ap()
    wqkv_d = nc.dram_tensor("wqkv", [C, 3 * C], bf16, kind="ExternalInput").ap()
    wout_d = nc.dram_tensor("wout", [C, C], bf16, kind="ExternalInput").ap()
    bias_d = nc.dram_tensor("biasT", [HEADS, N, N], bf16, kind="ExternalInput").ap()
    sel_d = nc.dram_tensor("sel", [2, 128], bf16, kind="ExternalInput").ap()
    out_d = nc.dram_tensor("out", [BPC, N, C], bf16, kind="ExternalOutput").ap()

    with tile.TileContext(nc) as tc, ExitStack() as ctx:
        persist = ctx.enter_context(tc.tile_pool(name="persist", bufs=1))
        xT_pool = ctx.enter_context(tc.tile_pool(name="xTp", bufs=2))
        e_pool = ctx.enter_context(tc.tile_pool(name="ep", bufs=1))
        attn_pool = ctx.enter_context(tc.tile_pool(name="atp", bufs=5))
        qkT_pool = ctx.enter_context(tc.tile_pool(name="qkTp", bufs=10))
        den_pool = ctx.enter_context(tc.tile_pool(name="dnp", bufs=1))
        osb_pool = ctx.enter_context(tc.tile_pool(name="osbp", bufs=2))
        sc_psum = ctx.enter_context(tc.tile_pool(name="scp", bufs=1, space="PSUM"))
        o_psum = ctx.enter_context(tc.tile_pool(name="opp", bufs=1, space="PSUM"))
        pj_psum = ctx.enter_context(tc.tile_pool(name="pjp", bufs=2, space="PSUM"))

        # ---- weights resident in SBUF ----
        # wqkv lands in two fused DMAs: columns 0:640 first (everything the
        # first attention pair needs: q tiles 0-3 and k tile 4) so the qkv
        # chain starts ~1.4MB into the HBM stream instead of 2.3MB.
        WA, WB = 640, 3 * C - 640
        wqkv_a = persist.tile([128, CT * WA], bf16, tag="wqkva")
        wqkv_b = persist.tile([128, CT * WB], bf16, tag="wqkvb")
        wout_sb = persist.tile([128, CT * C], bf16, tag="wout")

        def load_wqkv_a():
            nc.sync.dma_start(
                wqkv_a.rearrange("p (c k) -> p c k", c=CT),
                wqkv_d.rearrange("(c p) k -> p c k", p=128)[:, :, 0:WA])

        def load_wqkv_b():
            nc.sync.dma_start(
                wqkv_b.rearrange("p (c k) -> p c k", c=CT),
                wqkv_d.rearrange("(c p) k -> p c k", p=128)[:, :, WA:3 * C])

        def wslice(ci, k0, k1):
            # wqkv[ci*128:(ci+1)*128, k0:k1] from the split tiles
            if k1 <= WA:
                return wqkv_a[:, ci * WA + k0:ci * WA + k1]
            assert k0 >= WA
            return wqkv_b[:, ci * WB + k0 - WA:ci * WB + k1 - WA]

        def load_wout():
            nc.sync.dma_start(
                wout_sb.rearrange("p (c k) -> p c k", c=CT),
                wout_d.rearrange("(c p) k -> p c k", p=128))

        # ---- bias tables: one fat DMA per head, resident for both batches --
        bias_sb = {}

        def load_bias(h, eng=None):
            bt = persist.tile([TP, NT * N], bf16, tag=f"bias{h}",
                              name=f"bias{h}")
            (eng or nc.sync).dma_start(
                bt.rearrange("p (kt q) -> p kt q", kt=NT),
                bias_d[h].rearrange("(kt p) q -> p kt q", p=TP))
            bias_sb[h] = bt

        # warm up the ACT exp table load early (overlaps with qkv phase)
        warm = persist.tile([1, 16], f32, tag="warm")
        nc.vector.memset(warm, 0.0)
        nc.scalar.activation(warm, warm, EXP)

        # PE warm-up: dummy matmuls during the initial DMA wait keep the HAM
        # activity window busy so qkv starts at the full 2.4 GHz clock
        dmw = persist.tile([128, 256], bf16, tag="dmw")
        nc.vector.memset(dmw, 0.25)
        # selector for the PE-side recip broadcast: out[j, q] = db[j<64 ? 0:1, q]
        sel = persist.tile([2, 128], bf16, tag="sel")
        nc.sync.dma_start(sel, sel_d)
        _dn = [0]

        def emit_dummies(n, tag="d"):
            for i in range(n):
                _dn[0] += 1
                dp = pj_psum.tile([128, 512], f32, tag="pj",
                                  name=f"dmy{tag}_{_dn[0]}")
                nc.tensor.matmul(dp[0:64, 0:256], dmw[:, 0:64], dmw,
                                 start=True, stop=True)

        qkT, vsb, attnT, attn_sb, den_bf = {}, {}, {}, {}, {}
        for b in range(BPC):
            for fi in range(CT):
                attnT[b, fi] = persist.tile(
                    [128, N], bf16, tag=f"attnT{b}_{fi}", name=f"attnT{b}_{fi}")
            for pp in range(4):
                den_bf[b, pp] = persist.tile(
                    [2, N], bf16, tag=f"den{b}_{pp}", name=f"den{b}_{pp}")

        xts = {}

        def emit_xt(b):
            xt = xT_pool.tile([128, CT * N], bf16, tag="xT", name=f"xT{b}")
            nc.sync.dma_start(
                xt.rearrange("p (c q) -> p c q", c=CT),
                xT_d[b].rearrange("(c p) q -> p c q", p=128))
            xts[b] = xt

        def emit_qk_chunk(b, ft, ic):
            # half a qk feature tile (one q-chunk) -> finer filler granularity
            if (b, ft) not in qkT:
                qkT[b, ft] = qkT_pool.tile([128, N], bf16, tag="qkT",
                                           name=f"qkT{b}_{ft}")
            dst = qkT[b, ft]
            c0w, cw = CHUNKS[ic]
            ps = pj_psum.tile([128, 512], f32, tag="pj",
                              name=f"pj{b}_{ft}_{c0w}")
            for ci in range(CT):
                nc.tensor.matmul(
                    ps[:, 0:cw], wslice(ci, ft * 128, (ft + 1) * 128),
                    xts[b][:, ci * N + c0w:ci * N + c0w + cw],
                    start=(ci == 0), stop=(ci == CT - 1))
            nc.vector.tensor_copy(dst[:, c0w:c0w + cw], ps[:, 0:cw])

        def emit_qk_tile(b, ft):
            emit_qk_chunk(b, ft, 0)
            emit_qk_chunk(b, ft, 1)

        def emit_v_unit(b, t):
            vt = persist.tile([TP, HEADS, KD + 2], bf16, tag=f"v{b}_{t}",
                              name=f"v{b}_{t}")
            vsb[b, t] = vt
            ps = pj_psum.tile([128, 512], f32, tag="pj", name=f"pv{b}_{t}")
            for ci in range(CT):
                nc.tensor.matmul(
                    ps[0:TP, :], xts[b][:, ci * N + t * TP:ci * N + (t + 1) * TP],
                    wslice(ci, 2 * C, 3 * C),
                    start=(ci == 0), stop=(ci == CT - 1))
            nc.vector.tensor_copy(
                vt[:, :, 0:KD], ps[0:TP, :].rearrange("p (h d) -> p h d", h=HEADS))
            nc.vector.memset(vt[:, :, KD:KD + 2], 1.0)

        def emit_out_unit(b, t, scalar_copy=False):
            ps = pj_psum.tile([128, 512], f32, tag="pj", name=f"po{b}_{t}")
            for fi in range(CT):
                nc.tensor.matmul(
                    ps[0:TP, :], attnT[b, fi][:, t * TP:(t + 1) * TP],
                    wout_sb[:, fi * C:(fi + 1) * C],
                    start=(fi == 0), stop=(fi == CT - 1))
            osb = osb_pool.tile([TP, C], bf16, tag="osb")
            if scalar_copy:
                # final projection runs after all exps -- ACT is idle there
                nc.scalar.copy(osb, ps[0:TP, :])
            else:
                nc.vector.tensor_copy(osb, ps[0:TP, :])
            nc.sync.dma_start(out_d[b, t * TP:(t + 1) * TP, :], osb)

        pro_sc = {}

        def attn_prologue(b, pair):
            # first head-0 score tile of a pair; hoisted into the previous
            # pair's tail so the ACT pipeline never drains across pairs
            if (b, pair) in pro_sc:
                return
            t = sc_psum.tile([TP, 1024], f32, tag="sc0",
                             name=f"sc{b}_{pair}_0_0")
            pro_sc[b, pair] = t
            kT_t, qT_t = qkT[b, 4 + pair], qkT[b, pair]
            for (c0w, cw) in CHUNKS:
                nc.tensor.matmul(
                    t[:, c0w:c0w + cw],
                    kT_t[0:64, 0:TP], qT_t[0:64, c0w:c0w + cw],
                    start=True, stop=True)

        def attention(b, pair, fillers=(), nxt=None):
            fillers = list(fillers)
            h0, h1 = 2 * pair, 2 * pair + 1
            streams = ((0, h0), (1, h1))
            with nc.named_scope(f"attn_b{b}_p{pair}"):
                kT_t, qT_t = qkT[b, 4 + pair], qkT[b, pair]
                ops0, esbs, att, scs = {}, {}, {}, {}

                def sc_alloc(hs, kt):
                    t = sc_psum.tile([TP, 1024], f32, tag=f"sc{hs}",
                                     name=f"sc{b}_{pair}_{hs}_{kt}")
                    scs[hs, kt] = t
                    return t

                # Software-pipelined over kt: ACT(h0,kt) runs while the PE
                # writes scores for (h1,kt); ACT(h1,kt) covers (h0,kt+1).
                # ACT never waits on scores; scores never wait on ACT
                # (2 rotating psum tiles).
                attn_prologue(b, pair)
                scs[0, 0] = pro_sc[b, pair]
                for kt in range(NT):
                    # exp of h0's scores; bias-mult immediately behind it
                    esb0 = e_pool.tile([TP, N], bf16, tag=f"e0_{kt}",
                                       name=f"e{b}_{pair}_0_{kt}")
                    esbs[0, kt] = esb0
                    nc.scalar.activation(esb0, scs[0, kt][:, 0:N], EXP)
                    nc.vector.tensor_tensor(
                        esb0, esb0, bias_sb[h0][:, kt * N:(kt + 1) * N],
                        AluOpType.mult)
                    # h1's scores stream while ACT chews on h0
                    sc_alloc(1, kt)
                    for (c0w, cw) in CHUNKS:
                        nc.tensor.matmul(
                            scs[1, kt][:, c0w:c0w + cw],
                            kT_t[64:128, kt * TP:(kt + 1) * TP],
                            qT_t[64:128, c0w:c0w + cw],
                            start=True, stop=True)
                    esb1 = e_pool.tile([TP, N], bf16, tag=f"e1_{kt}",
                                       name=f"e{b}_{pair}_1_{kt}")
                    esbs[1, kt] = esb1
                    nc.scalar.activation(esb1, scs[1, kt][:, 0:N], EXP)
                    nc.gpsimd.tensor_tensor(
                        esb1, esb1, bias_sb[h1][:, kt * N:(kt + 1) * N],
                        AluOpType.mult)
                    # chunk-0 v-matmul trails by two kt so its bias-multiply
                    # has two iterations of slack (covers the slower GPSIMD
                    # tensor_tensor on the offloaded head)
                    if kt >= 2:
                        for hs, h in streams:
                            if kt == 2:
                                op = o_psum.tile([128, 512], f32,
                                                 tag=f"op{hs}",
                                                 name=f"op0_{b}_{h}")
                                ops0[hs] = op[0:KD + 1, :]
                            nc.tensor.matmul(
                                ops0[hs], vsb[b, kt - 2][:, h, 0:KD + 1],
                                esbs[hs, kt - 2][:, 0:512],
                                start=(kt == 2), stop=False)
                    # h0's scores for kt+1 run while ACT chews on h1
                    if kt + 1 < NT:
                        sc_alloc(0, kt + 1)
                        for (c0w, cw) in CHUNKS:
                            nc.tensor.matmul(
                                scs[0, kt + 1][:, c0w:c0w + cw],
                                kT_t[0:64, (kt + 1) * TP:(kt + 2) * TP],
                                qT_t[0:64, c0w:c0w + cw],
                                start=True, stop=True)
                    if fillers:
                        fillers.pop(0)()
                for ktv in (NT - 2, NT - 1):
                    for hs, h in streams:
                        nc.tensor.matmul(
                            ops0[hs], vsb[b, ktv][:, h, 0:KD + 1],
                            esbs[hs, ktv][:, 0:512],
                            start=False, stop=(ktv == NT - 1))
                if nxt is not None:
                    attn_prologue(*nxt)
                for hs, h in streams:
                    a = attn_pool.tile([KD + 1, N], bf16, tag="attn",
                                       name=f"attn{b}_{h}")
                    att[hs] = a
                    attn_sb[b, h] = a
                    nc.vector.tensor_copy(a[:, 0:512], ops0[hs])
                # chunk-1 accumulators from the pj pool: decouples these vMMs
                # from the chunk-0 copy's o_psum slot release.  kt-interleaved
                # across both heads so e{hs}_0/e{hs}_1 release early and the
                # next pair's ACT pipeline refills without draining this tail.
                ops1 = {}
                for hs, h in streams:
                    ops1[hs] = pj_psum.tile([KD + 1, 512], f32, tag="pj",
                                            name=f"op1_{b}_{h}")
                for kt in range(NT):
                    for hs, h in streams:
                        nc.tensor.matmul(
                            ops1[hs][:, 0:272], vsb[b, kt][:, h, 0:KD + 1],
                            esbs[hs, kt][:, 512:784],
                            start=(kt == 0), stop=(kt == NT - 1))
                for hs, h in streams:
                    nc.vector.tensor_copy(att[hs][:, 512:784],
                                          ops1[hs][:, 0:272])
                for hs, h in streams:
                    nc.sync.dma_start(den_bf[b, pair][hs:hs + 1, :],
                                      att[hs][KD:KD + 1, :])
                while fillers:
                    fillers.pop(0)()

        dbs = {}

        def norm_recip(b, pair):
            # DVE-only piece of the normalization (no PE involvement)
            with nc.named_scope(f"nrecip_b{b}_{pair}"):
                dc = den_pool.tile([2, N], f32, tag="dc")
                nc.vector.tensor_copy(dc, den_bf[b, pair])
                dr = den_pool.tile([2, N], f32, tag="dr")
                nc.vector.reciprocal_approx_fast(dr, dc)
                db = den_pool.tile([2, N], bf16, tag="db")
                nc.vector.tensor_copy(db, dr)
                dbs[b, pair] = db

        def norm_apply(b, pair):
            # attnT[b,pair] = attn / den: the reciprocal rows are broadcast
            # across 128 partitions by a tiny PE matmul against the selector
            # (a stride-0 broadcast DMA here costs ~5us of descriptor
            # processing and head-of-line-blocks the DVE queue).  Must be
            # emitted >=2 filler slots after norm_recip so the PE never waits
            # on the DVE recip chain.
            from concourse.alu_op_type import AluOpType
            db = dbs[b, pair]
            with nc.named_scope(f"napply_b{b}_{pair}"):
                for ic, (c0w, cw) in enumerate(CHUNKS):
                    rp = pj_psum.tile([128, 512], f32, tag="pj",
                                      name=f"rb{b}_{pair}_{c0w}")
                    nc.tensor.matmul(rp[:, 0:cw], sel, db[0:2, c0w:c0w + cw],
                                     start=True, stop=True)
                    for hs in range(2):
                        h = 2 * pair + hs
                        r0 = hs * 64
                        nc.vector.tensor_tensor(
                            attnT[b, pair][r0:r0 + 64, c0w:c0w + cw],
                            attn_sb[b, h][0:KD, c0w:c0w + cw],
                            rp[r0:r0 + 64, 0:cw], AluOpType.mult)

        def norm_pair(b, pair):
            norm_recip(b, pair)
            norm_apply(b, pair)

        # ---- schedule (b-major; fillers keep the PE warm during attention) --
        def qkh(b, ft, ic):
            return lambda: emit_qk_chunk(b, ft, ic)

        def vu(b, t):
            return lambda: emit_v_unit(b, t)

        def ou(b, t):
            return lambda: emit_out_unit(b, t)

        def dmy():
            return lambda: emit_dummies(1)

        def rcp(b, pair):
            return lambda: norm_recip(b, pair)

        def app(b, pair):
            return lambda: norm_apply(b, pair)

        emit_xt(0)
        load_wqkv_a()
        load_wqkv_b()
        load_bias(0)
        load_bias(1)
        emit_dummies(32, "w")          # PE warm-up during the initial DMAs
        with nc.named_scope("qkv_early_b0"):
            emit_qk_tile(0, 0)
            emit_qk_tile(0, 4)
            attn_prologue(0, 0)
            emit_v_unit(0, 0)
            emit_v_unit(0, 1)
            emit_v_unit(0, 2)
        load_bias(2)
        load_bias(3)
        attention(0, 0, [qkh(0, 1, 0), qkh(0, 1, 1), qkh(0, 5, 0),
                         vu(0, 3), vu(0, 4), vu(0, 5), vu(0, 6)],
                  nxt=(0, 1))
        load_wout()
        load_bias(4)
        load_bias(5)
        emit_xt(1)
        attention(0, 1, [qkh(0, 5, 1), qkh(0, 2, 0), qkh(0, 2, 1),
                         qkh(0, 6, 0), vu(1, 0), vu(1, 1), vu(1, 2)],
                  nxt=(0, 2))
        load_bias(6)
        load_bias(7)
        norm_pair(0, 0)
        attention(0, 2, [qkh(0, 6, 1), qkh(0, 3, 0), qkh(0, 3, 1),
                         qkh(0, 7, 0), vu(1, 3), vu(1, 4), vu(1, 5)],
                  nxt=(0, 3))
        norm_pair(0, 1)
        norm_pair(0, 2)
        attention(0, 3, [qkh(0, 7, 1), qkh(1, 0, 0), qkh(1, 0, 1),
                         qkh(1, 4, 0), vu(1, 6), dmy(), dmy()],
                  nxt=(1, 0))
        norm_pair(0, 3)
        attention(1, 0, [qkh(1, 4, 1), qkh(1, 1, 0), qkh(1, 1, 1),
                         qkh(1, 5, 0), ou(0, 0), ou(0, 1), dmy()],
                  nxt=(1, 1))
        norm_pair(1, 0)
        attention(1, 1, [qkh(1, 5, 1), qkh(1, 2, 0), qkh(1, 2, 1),
                         qkh(1, 6, 0), ou(0, 2), ou(0, 3), dmy()],
                  nxt=(1, 2))
        norm_pair(1, 1)
        attention(1, 2, [qkh(1, 6, 1), qkh(1, 3, 0), qkh(1, 3, 1),
                         qkh(1, 7, 0), ou(0, 4), ou(0, 5), dmy()],
                  nxt=(1, 3))
        norm_pair(1, 2)
        attention(1, 3, [qkh(1, 7, 1), ou(0, 6), dmy(), dmy(), dmy(), dmy()])
        # keep the PE warm through the last norm chain so proj_b1 runs at
        # full clock, and emit the apply two dummy-batches after the recip
        # so the PE FIFO never waits on the DVE
        emit_dummies(4, "t1")
        norm_recip(1, 3)
        emit_dummies(8, "t2")
        norm_apply(1, 3)
        with nc.named_scope("proj_b1"):
            for t in range(NT):
                emit_out_unit(1, t, scalar_copy=True)

    nc.compile()
    _CACHE['nc'] = nc
    return nc


def host_prep(x, w_qkv, pos_table, w_out):
    x = np.asarray(x, np.float32).reshape(B, N, C)
    wq = np.array(np.asarray(w_qkv, np.float32), copy=True)
    wq[:, :C] *= np.float32(1.0 / np.sqrt(KD))
    wq_bf = wq.astype(ml_dtypes.bfloat16)
    idx = _rel_index()
    biasT = np.ascontiguousarray(np.exp(
        np.asarray(pos_table, np.float32)[:, idx].transpose(0, 2, 1)
    )).astype(ml_dtypes.bfloat16)
    wout = np.ascontiguousarray(np.asarray(w_out, np.float32)).astype(
        ml_dtypes.bfloat16)
    sel = np.zeros((2, 128), np.float32)
    sel[0, 0:64] = 1.0
    sel[1, 64:128] = 1.0
    sel = sel.astype(ml_dtypes.bfloat16)
    in_maps = []
    for c in range(NCORES):
        xT = np.ascontiguousarray(
            x[c * BPC:(c + 1) * BPC].transpose(0, 2, 1)).astype(
                ml_dtypes.bfloat16)  # [2, 512, 784]
        in_maps.append({"xT": xT, "wqkv": wq_bf, "wout": wout,
                        "biasT": biasT, "sel": sel})
    return in_maps


def run(in_maps, trace=False, trace_cores=None):
    import concourse.bass_utils as bass_utils
    nc = build_nc()
    return bass_utils.run_bass_kernel_spmd(
        nc, in_maps, core_ids=list(range(NCORES)),
        trace=trace, trace_cores=trace_cores)


def kernel(x, w_qkv, pos_table, w_out):
    in_maps = host_prep(x, w_qkv, pos_table, w_out)
    res = run(in_maps)
    out = np.stack([np.asarray(r["out"], np.float32) for r in res.results])
    return np.ascontiguousarray(out.reshape(B, HH, WW, C))


# revision 63
# speedup vs baseline: 1.1152x; 1.1152x over previous
"""Trainium2 Bass kernel: MHSA with multi-head relative position embedding.

Sharding: data-parallel over batch — 16 batches / 8 cores = 2 batches per core,
each core computes all 8 heads for its 2 batches. No collectives needed.

Math per batch (N=784 tokens, C=512, 8 heads x 64 dim):
  qkv = x @ w_qkv                  (q-columns pre-scaled by 1/8 on host)
  scores_T[k,q] = k_h^T q_h        (head pair packed in one 4-bank PSUM tile;
                                    even/odd head matmuls issued adjacently so
                                    they pack into disjoint PE row groups)
  E = exp(scores_T) * expbias_T    (ONE fused exp on ACT covering both heads'
                                    784-wide rows via a strided AP; bias
                                    multiply split DVE/GPSIMD, bias exp'd host)
  O_T[d,q] = sum_k v_aug[k,d] E[k,q]  with v_aug = [v | 1] -> row 64 = sumexp
  attnT = O_T[0:64] / O_T[64]      (denominator rows DMA-gathered, recip'd,
                                    replicated to 128 partitions with a
                                    stride-0 DMA, DVE mult)
  out = attnT^T stacked over heads @ w_out   (written bf16, host casts f32)

Perf structure vs v1: bias tables loaded ONCE (8 fat DMAs, resident in SBUF
for both batches) instead of per-batch per-tile; PE warmed with dummy matmuls
during the initial DMA wait so qkv runs at full clock (HAM 8/8); exp fused to
one ACTIVATE per (pair, kt) halving ACT instruction overhead; bias multiplies
split ~50/50 between DVE and GPSIMD; projection fillers redistributed so the
PE never idles >2us (keeps the HAM clock-gate warm).
"""

import numpy as np
import ml_dtypes

B, HH, WW, C = 16, 28, 28, 512
N = HH * WW            # 784 tokens
HEADS, KD = 8, 64
NCORES, BPC = 8, 2     # 8 cores, 2 batches per core
NT, TP = 7, 112        # 784 = 7 tiles of 112 (k / token tiling)
CHUNKS = [(0, 512), (512, 272)]   # q-chunks (PSUM bank = 512 fp32)
CT = 4                 # contraction tiles of 128 over C=512

_CACHE = {}


def _rel_index():
    # Faithful to reference._relative_position_index: token r -> (r%28, r//28)
    t = np.arange(N)
    c0, c1 = t % HH, t // HH
    return ((c0[:, None] - c0[None, :] + HH - 1)
            + (c1[:, None] - c1[None, :] + WW - 1) * (2 * HH - 1))  # [q, k]


def build_nc():
    if 'nc' in _CACHE:
        return _CACHE['nc']
    from contextlib import ExitStack
    import concourse.bacc as bacc
    import concourse.mybir as mybir
    import concourse.tile as tile
    from concourse.alu_op_type import AluOpType

    f32 = mybir.dt.float32
    bf16 = mybir.dt.bfloat16
    EXP = mybir.ActivationFunctionType.Exp

    nc = bacc.Bacc("TRN2", debug=False, enable_asserts=False)
    xT_d = nc.dram_tensor("xT", [BPC, C, N], bf16, kind="ExternalInput").ap()
    wqkv_d = nc.dram_tensor("wqkv", [C, 3 * C], bf16, kind="ExternalInput").ap()
    wout_d = nc.dram_tensor("wout", [C, C], bf16, kind="ExternalInput").ap()
    bias_d = nc.dram_tensor("biasT", [HEADS, N, N], bf16, kind="ExternalInput").ap()
    sel_d = nc.dram_tensor("sel", [2, 128], bf16, kind="ExternalInput").ap()
    out_d = nc.dram_tensor("out", [BPC, N, C], bf16, kind="ExternalOutput").ap()

    with tile.TileContext(nc) as tc, ExitStack() as ctx:
        persist = ctx.enter_context(tc.tile_pool(name="persist", bufs=1))
        xT_pool = ctx.enter_context(tc.tile_pool(name="xTp", bufs=2))
        e_pool = ctx.enter_context(tc.tile_pool(name="ep", bufs=1))
        attn_pool = ctx.enter_context(tc.tile_pool(name="atp", bufs=5))
        qkT_pool = ctx.enter_context(tc.tile_pool(name="qkTp", bufs=10))
        den_pool = ctx.enter_context(tc.tile_pool(name="dnp", bufs=1))
        osb_pool = ctx.enter_context(tc.tile_pool(name="osbp", bufs=2))
        sc_psum = ctx.enter_context(tc.tile_pool(name="scp", bufs=1, space="PSUM"))
        o_psum = ctx.enter_context(tc.tile_pool(name="opp", bufs=1, space="PSUM"))
        pj_psum = ctx.enter_context(tc.tile_pool(name="pjp", bufs=2, space="PSUM"))

        # ---- weights resident in SBUF ----
        # wqkv lands in two fused DMAs: columns 0:640 first (everything the
        # first attention pair needs: q tiles 0-3 and k tile 4) so the qkv
        # chain starts ~1.4MB into the HBM stream instead of 2.3MB.
        WA, WB = 640, 3 * C - 640
        wqkv_a = persist.tile([128, CT * WA], bf16, tag="wqkva")
        wqkv_b = persist.tile([128, CT * WB], bf16, tag="wqkvb")
        wout_sb = persist.tile([128, CT * C], bf16, tag="wout")

        def load_wqkv_a():
            nc.sync.dma_start(
                wqkv_a.rearrange("p (c k) -> p c k", c=CT),
                wqkv_d.rearrange("(c p) k -> p c k", p=128)[:, :, 0:WA])

        def load_wqkv_b():
            nc.sync.dma_start(
                wqkv_b.rearrange("p (c k) -> p c k", c=CT),
                wqkv_d.rearrange("(c p) k -> p c k", p=128)[:, :, WA:3 * C])

        def wslice(ci, k0, k1):
            # wqkv[ci*128:(ci+1)*128, k0:k1] from the split tiles
            if k1 <= WA:
                return wqkv_a[:, ci * WA + k0:ci * WA + k1]
            assert k0 >= WA
            return wqkv_b[:, ci * WB + k0 - WA:ci * WB + k1 - WA]

        def load_wout():
            nc.sync.dma_start(
                wout_sb.rearrange("p (c k) -> p c k", c=CT),
                wout_d.rearrange("(c p) k -> p c k", p=128))

        # ---- bias tables: one fat DMA per head, resident for both batches --
        bias_sb = {}

        def load_bias(h, eng=None):
            bt = persist.tile([TP, NT * N], bf16, tag=f"bias{h}",
                              name=f"bias{h}")
            (eng or nc.sync).dma_start(
                bt.rearrange("p (kt q) -> p kt q", kt=NT),
                bias_d[h].rearrange("(kt p) q -> p kt q", p=TP))
            bias_sb[h] = bt

        # warm up the ACT exp table load early (overlaps with qkv phase)
        warm = persist.tile([1, 16], f32, tag="warm")
        nc.vector.memset(warm, 0.0)
        nc.scalar.activation(warm, warm, EXP)

        # PE warm-up: dummy matmuls during the initial DMA wait keep the HAM
        # activity window busy so qkv starts at the full 2.4 GHz clock
        dmw = persist.tile([128, 256], bf16, tag="dmw")
        nc.vector.memset(dmw, 0.25)
        # selector for the PE-side recip broadcast: out[j, q] = db[j<64 ? 0:1, q]
        sel = persist.tile([2, 128], bf16, tag="sel")
        nc.sync.dma_start(sel, sel_d)
        _dn = [0]

        def emit_dummies(n, tag="d"):
            for i in range(n):
                _dn[0] += 1
                dp = pj_psum.tile([128, 512], f32, tag="pj",
                                  name=f"dmy{tag}_{_dn[0]}")
                nc.tensor.matmul(dp[0:64, 0:256], dmw[:, 0:64], dmw,
                                 start=True, stop=True)

        qkT, vsb, attnT, attn_sb, den_bf = {}, {}, {}, {}, {}
        for b in range(BPC):
            for fi in range(CT):
                attnT[b, fi] = persist.tile(
                    [128, N], bf16, tag=f"attnT{b}_{fi}", name=f"attnT{b}_{fi}")
            for pp in range(4):
                den_bf[b, pp] = persist.tile(
                    [2, N], bf16, tag=f"den{b}_{pp}", name=f"den{b}_{pp}")

        xts = {}

        def emit_xt(b):
            xt = xT_pool.tile([128, CT * N], bf16, tag="xT", name=f"xT{b}")
            nc.sync.dma_start(
                xt.rearrange("p (c q) -> p c q", c=CT),
                xT_d[b].rearrange("(c p) q -> p c q", p=128))
            xts[b] = xt

        def emit_qk_chunk(b, ft, ic):
            # half a qk feature tile (one q-chunk) -> finer filler granularity
            if (b, ft) not in qkT:
                qkT[b, ft] = qkT_pool.tile([128, N], bf16, tag="qkT",
                                           name=f"qkT{b}_{ft}")
            dst = qkT[b, ft]
            c0w, cw = CHUNKS[ic]
            ps = pj_psum.tile([128, 512], f32, tag="pj",
                              name=f"pj{b}_{ft}_{c0w}")
            for ci in range(CT):
                nc.tensor.matmul(
                    ps[:, 0:cw], wslice(ci, ft * 128, (ft + 1) * 128),
                    xts[b][:, ci * N + c0w:ci * N + c0w + cw],
                    start=(ci == 0), stop=(ci == CT - 1))
            nc.vector.tensor_copy(dst[:, c0w:c0w + cw], ps[:, 0:cw])

        def emit_qk_tile(b, ft):
            emit_qk_chunk(b, ft, 0)
            emit_qk_chunk(b, ft, 1)

        def emit_v_unit(b, t):
            vt = persist.tile([TP, HEADS, KD + 2], bf16, tag=f"v{b}_{t}",
                              name=f"v{b}_{t}")
            vsb[b, t] = vt
            ps = pj_psum.tile([128, 512], f32, tag="pj", name=f"pv{b}_{t}")
            for ci in range(CT):
                nc.tensor.matmul(
                    ps[0:TP, :], xts[b][:, ci * N + t * TP:ci * N + (t + 1) * TP],
                    wslice(ci, 2 * C, 3 * C),
                    start=(ci == 0), stop=(ci == CT - 1))
            nc.vector.tensor_copy(
                vt[:, :, 0:KD], ps[0:TP, :].rearrange("p (h d) -> p h d", h=HEADS))
            nc.vector.memset(vt[:, :, KD:KD + 2], 1.0)

        def emit_out_unit(b, t, scalar_copy=False):
            ps = pj_psum.tile([128, 512], f32, tag="pj", name=f"po{b}_{t}")
            for fi in range(CT):
                nc.tensor.matmul(
                    ps[0:TP, :], attnT[b, fi][:, t * TP:(t + 1) * TP],
                    wout_sb[:, fi * C:(fi + 1) * C],
                    start=(fi == 0), stop=(fi == CT - 1))
            osb = osb_pool.tile([TP, C], bf16, tag="osb")
            if scalar_copy:
                # final projection runs after all exps -- ACT is idle there
                nc.scalar.copy(osb, ps[0:TP, :])
            else:
                nc.vector.tensor_copy(osb, ps[0:TP, :])
            nc.sync.dma_start(out_d[b, t * TP:(t + 1) * TP, :], osb)

        pro_sc = {}

        def attn_prologue(b, pair):
            # first head-0 score tile of a pair; hoisted into the previous
            # pair's tail so the ACT pipeline never drains across pairs
            if (b, pair) in pro_sc:
                return
            t = sc_psum.tile([TP, 1024], f32, tag="sc0",
                             name=f"sc{b}_{pair}_0_0")
            pro_sc[b, pair] = t
            kT_t, qT_t = qkT[b, 4 + pair], qkT[b, pair]
            for (c0w, cw) in CHUNKS:
                nc.tensor.matmul(
                    t[:, c0w:c0w + cw],
                    kT_t[0:64, 0:TP], qT_t[0:64, c0w:c0w + cw],
                    start=True, stop=True)

        def attention(b, pair, fillers=(), nxt=None):
            fillers = list(fillers)
            h0, h1 = 2 * pair, 2 * pair + 1
            streams = ((0, h0), (1, h1))
            with nc.named_scope(f"attn_b{b}_p{pair}"):
                kT_t, qT_t = qkT[b, 4 + pair], qkT[b, pair]
                ops0, esbs, att, scs = {}, {}, {}, {}

                def sc_alloc(hs, kt):
                    t = sc_psum.tile([TP, 1024], f32, tag=f"sc{hs}",
                                     name=f"sc{b}_{pair}_{hs}_{kt}")
                    scs[hs, kt] = t
                    return t

                # Software-pipelined over kt: ACT(h0,kt) runs while the PE
                # writes scores for (h1,kt); ACT(h1,kt) covers (h0,kt+1).
                # ACT never waits on scores; scores never wait on ACT
                # (2 rotating psum tiles).
                attn_prologue(b, pair)
                scs[0, 0] = pro_sc[b, pair]
                for kt in range(NT):
                    # exp of h0's scores; bias-mult immediately behind it
                    esb0 = e_pool.tile([TP, N], bf16, tag=f"e0_{kt}",
                                       name=f"e{b}_{pair}_0_{kt}")
                    esbs[0, kt] = esb0
                    nc.scalar.activation(esb0, scs[0, kt][:, 0:N], EXP)
                    nc.vector.tensor_tensor(
                        esb0, esb0, bias_sb[h0][:, kt * N:(kt + 1) * N],
                        AluOpType.mult)
                    # h1's scores stream while ACT chews on h0
                    sc_alloc(1, kt)
                    for (c0w, cw) in CHUNKS:
                        nc.tensor.matmul(
                            scs[1, kt][:, c0w:c0w + cw],
                            kT_t[64:128, kt * TP:(kt + 1) * TP],
                            qT_t[64:128, c0w:c0w + cw],
                            start=True, stop=True)
                    esb1 = e_pool.tile([TP, N], bf16, tag=f"e1_{kt}",
                                       name=f"e{b}_{pair}_1_{kt}")
                    esbs[1, kt] = esb1
                    nc.scalar.activation(esb1, scs[1, kt][:, 0:N], EXP)
                    nc.gpsimd.tensor_tensor(
                        esb1, esb1, bias_sb[h1][:, kt * N:(kt + 1) * N],
                        AluOpType.mult)
                    # chunk-0 v-matmul trails by two kt so its bias-multiply
                    # has two iterations of slack (covers the slower GPSIMD
                    # tensor_tensor on the offloaded head)
                    if kt >= 2:
                        for hs, h in streams:
                            if kt == 2:
                                op = o_psum.tile([128, 512], f32,
                                                 tag=f"op{hs}",
                                                 name=f"op0_{b}_{h}")
                                ops0[hs] = op[0:KD + 1, :]
                            nc.tensor.matmul(
                                ops0[hs], vsb[b, kt - 2][:, h, 0:KD + 1],
                                esbs[hs, kt - 2][:, 0:512],
                                start=(kt == 2), stop=False)
                    # h0's scores for kt+1 run while ACT chews on h1
                    if kt + 1 < NT:
                        sc_alloc(0, kt + 1)
                        for (c0w, cw) in CHUNKS:
                            nc.tensor.matmul(
                                scs[0, kt + 1][:, c0w:c0w + cw],
                                kT_t[0:64, (kt + 1) * TP:(kt + 2) * TP],
                                qT_t[0:64, c0w:c0w + cw],
                                start=True, stop=True)
                    if fillers:
                        fillers.pop(0)()
                for ktv in (NT - 2, NT - 1):
                    for hs, h in streams:
                        nc.tensor.matmul(
                            ops0[hs], vsb[b, ktv][:, h, 0:KD + 1],
                            esbs[hs, ktv][:, 0:512],
                            start=False, stop=(ktv == NT - 1))
                if nxt is not None:
                    attn_prologue(*nxt)
                for hs, h in streams:
                    a = attn_pool.tile([KD + 1, N], bf16, tag="attn",
                                       name=f"attn{b}_{h}")
                    att[hs] = a
                    attn_sb[b, h] = a
                    nc.vector.tensor_copy(a[:, 0:512], ops0[hs])
                # chunk-1 accumulators from the pj pool: decouples these vMMs
                # from the chunk-0 copy's o_psum slot release.  kt-interleaved
                # across both heads so e{hs}_0/e{hs}_1 release early and the
                # next pair's ACT pipeline refills without draining this tail.
                ops1 = {}
                for hs, h in streams:
                    ops1[hs] = pj_psum.tile([KD + 1, 512], f32, tag="pj",
                                            name=f"op1_{b}_{h}")
                for kt in range(NT):
                    for hs, h in streams:
                        nc.tensor.matmul(
                            ops1[hs][:, 0:272], vsb[b, kt][:, h, 0:KD + 1],
                            esbs[hs, kt][:, 512:784],
                            start=(kt == 0), stop=(kt == NT - 1))
                for hs, h in streams:
                    nc.vector.tensor_copy(att[hs][:, 512:784],
                                          ops1[hs][:, 0:272])
                for hs, h in streams:
                    nc.sync.dma_start(den_bf[b, pair][hs:hs + 1, :],
                                      att[hs][KD:KD + 1, :])
                while fillers:
                    fillers.pop(0)()

        dbs = {}

        def norm_recip(b, pair):
            # DVE-only piece of the normalization (no PE involvement)
            with nc.named_scope(f"nrecip_b{b}_{pair}"):
                dc = den_pool.tile([2, N], f32, tag="dc")
                nc.vector.tensor_copy(dc, den_bf[b, pair])
                dr = den_pool.tile([2, N], f32, tag="dr")
                nc.vector.reciprocal_approx_fast(dr, dc)
                db = den_pool.tile([2, N], bf16, tag="db")
                nc.vector.tensor_copy(db, dr)
                dbs[b, pair] = db

        def norm_apply(b, pair):
            # attnT[b,pair] = attn / den: the reciprocal rows are broadcast
            # across 128 partitions by a tiny PE matmul against the selector
            # (a stride-0 broadcast DMA here costs ~5us of descriptor
            # processing and head-of-line-blocks the DVE queue).  Must be
            # emitted >=2 filler slots after norm_recip so the PE never waits
            # on the DVE recip chain.
            from concourse.alu_op_type import AluOpType
            db = dbs[b, pair]
            with nc.named_scope(f"napply_b{b}_{pair}"):
                for ic, (c0w, cw) in enumerate(CHUNKS):
                    # o_psum slots are idle between pairs; using them (not the
                    # pj pool) keeps the filler rotation unblocked.  Only safe
                    # OUTSIDE an attention kt-loop (in-loop it deadlocks
                    # against the held chunk-0 accumulators).
                    rp = o_psum.tile([128, 512], f32, tag=f"op{ic}",
                                     name=f"rb{b}_{pair}_{c0w}")
                    nc.tensor.matmul(rp[:, 0:cw], sel, db[0:2, c0w:c0w + cw],
                                     start=True, stop=True)
                    for hs in range(2):
                        h = 2 * pair + hs
                        r0 = hs * 64
                        nc.vector.tensor_tensor(
                            attnT[b, pair][r0:r0 + 64, c0w:c0w + cw],
                            attn_sb[b, h][0:KD, c0w:c0w + cw],
                            rp[r0:r0 + 64, 0:cw], AluOpType.mult)

        def norm_pair(b, pair):
            norm_recip(b, pair)
            norm_apply(b, pair)

        # ---- schedule (b-major; fillers keep the PE warm during attention) --
        def qkh(b, ft, ic):
            return lambda: emit_qk_chunk(b, ft, ic)

        def vu(b, t):
            return lambda: emit_v_unit(b, t)

        def ou(b, t):
            return lambda: emit_out_unit(b, t)

        def dmy():
            return lambda: emit_dummies(1)

        def rcp(b, pair):
            return lambda: norm_recip(b, pair)

        def app(b, pair):
            return lambda: norm_apply(b, pair)

        emit_xt(0)
        load_wqkv_a()
        load_wqkv_b()
        load_bias(0)
        load_bias(1)
        emit_dummies(32, "w")          # PE warm-up during the initial DMAs
        with nc.named_scope("qkv_early_b0"):
            emit_qk_tile(0, 0)
            emit_qk_tile(0, 4)
            attn_prologue(0, 0)
            emit_v_unit(0, 0)
            emit_v_unit(0, 1)
            emit_v_unit(0, 2)
        load_bias(2)
        load_bias(3)
        attention(0, 0, [qkh(0, 1, 0), qkh(0, 1, 1), qkh(0, 5, 0),
                         vu(0, 3), vu(0, 4), vu(0, 5), vu(0, 6)],
                  nxt=(0, 1))
        load_wout()
        load_bias(4)
        load_bias(5)
        emit_xt(1)
        # Norm chains staggered around whole pairs: recip (DVE-only) right
        # after its pair's denominators land, apply (PE broadcast + DVE mult)
        # after the NEXT pair -- the PE FIFO and psum pools never wait on a
        # fresh denominator, and attn tiles are read before buffer reuse.
        norm_recip(0, 0)
        attention(0, 1, [qkh(0, 5, 1), qkh(0, 2, 0), qkh(0, 2, 1),
                         qkh(0, 6, 0), vu(1, 0), vu(1, 1), vu(1, 2)],
                  nxt=(0, 2))
        load_bias(6)
        load_bias(7)
        norm_apply(0, 0)
        norm_recip(0, 1)
        attention(0, 2, [qkh(0, 6, 1), qkh(0, 3, 0), qkh(0, 3, 1),
                         qkh(0, 7, 0), vu(1, 3), vu(1, 4), vu(1, 5)],
                  nxt=(0, 3))
        norm_apply(0, 1)
        norm_recip(0, 2)
        attention(0, 3, [qkh(0, 7, 1), qkh(1, 0, 0), qkh(1, 0, 1),
                         qkh(1, 4, 0), vu(1, 6), dmy(), dmy()],
                  nxt=(1, 0))
        norm_apply(0, 2)
        norm_recip(0, 3)
        attention(1, 0, [qkh(1, 4, 1), qkh(1, 1, 0), qkh(1, 1, 1),
                         qkh(1, 5, 0), dmy(), dmy(), dmy()],
                  nxt=(1, 1))
        norm_apply(0, 3)
        norm_recip(1, 0)
        attention(1, 1, [qkh(1, 5, 1), qkh(1, 2, 0), qkh(1, 2, 1),
                         qkh(1, 6, 0), ou(0, 0), ou(0, 1), dmy()],
                  nxt=(1, 2))
        norm_apply(1, 0)
        norm_recip(1, 1)
        attention(1, 2, [qkh(1, 6, 1), qkh(1, 3, 0), qkh(1, 3, 1),
                         qkh(1, 7, 0), ou(0, 2), ou(0, 3), dmy()],
                  nxt=(1, 3))
        norm_apply(1, 1)
        norm_recip(1, 2)
        attention(1, 3, [qkh(1, 7, 1), ou(0, 4), ou(0, 5), ou(0, 6),
                         dmy(), dmy()])
        # keep the PE warm through the last norm chains so proj_b1 runs at
        # full clock; recip and apply separated by dummy batches so the PE
        # FIFO never waits on the DVE
        norm_apply(1, 2)
        emit_dummies(4, "t1")
        norm_recip(1, 3)
        emit_dummies(8, "t2")
        norm_apply(1, 3)
        with nc.named_scope("proj_b1"):
            for t in range(NT):
                emit_out_unit(1, t, scalar_copy=True)

    nc.compile()
    _CACHE['nc'] = nc
    return nc


def host_prep(x, w_qkv, pos_table, w_out):
    x = np.asarray(x, np.float32).reshape(B, N, C)
    wq = np.array(np.asarray(w_qkv, np.float32), copy=True)
    wq[:, :C] *= np.float32(1.0 / np.sqrt(KD))
    wq_bf = wq.astype(ml_dtypes.bfloat16)
    idx = _rel_index()
    biasT = np.ascontiguousarray(np.exp(
        np.asarray(pos_table, np.float32)[:, idx].transpose(0, 2, 1)
    )).astype(ml_dtypes.bfloat16)
    wout = np.ascontiguousarray(np.asarray(w_out, np.float32)).astype(
        ml_dtypes.bfloat16)
    sel = np.zeros((2, 128), np.float32)
    sel[0, 0:64] = 1.0
    sel[1, 64:128] = 1.0
    sel = sel.astype(ml_dtypes.bfloat16)
    in_maps = []
    for c in range(NCORES):
        xT = np.ascontiguousarray(
            x[c * BPC:(c + 1) * BPC].transpose(0, 2, 1)).astype(
                ml_dtypes.bfloat16)  # [2, 512, 784]
        in_maps.append({"xT": xT, "wqkv": wq_bf, "wout": wout,
                        "biasT": biasT, "sel": sel})
    return in_maps


def run(in_maps, trace=False, trace_cores=None):
    import concourse.bass_utils as bass_utils
    nc = build_nc()
    return bass_utils.run_bass_kernel_spmd(
        nc, in_maps, core_ids=list(range(NCORES)),
        trace=trace, trace_cores=trace_cores)


def kernel(x, w_qkv, pos_table, w_out):
    in_maps = host_prep(x, w_qkv, pos_table, w_out)
    res = run(in_maps)
    out = np.stack([np.asarray(r["out"], np.float32) for r in res.results])
    return np.ascontiguousarray(out.reshape(B, HH, WW, C))
